# revision 4
# baseline (speedup 1.0000x reference)
"""Trainium2 Bass kernel for nn_BottleNeckKAGNConvNDLayer.

Math (per image x[512, 64, 64], G=1):
  basis = pw(dw(silu(x)))                       # depthwise 3x3 + pointwise
  xi    = tanh(inner @ x)                       # 512 -> 128 bottleneck
  gb    = silu([1, xi, xi^2-bt2, xi*(xi^2-bt2-bt3)])   # Gram basis, 512 ch
  y     = out_w @ conv3x3(gb, poly_w)           # 512 -> 128 -> 512
  out   = silu(instance_norm(y + basis) * gamma + beta)

Distribution: data-parallel over the 16 images (8 rgb + 8 ir) across 8
cores, 2 images per core; weights replicated. No collectives.

On-chip layout: channels on partitions (4 tiles of 128 for 512-ch
tensors). Spatial 64x64 stored zero-padded as 66 rows x 68 cols
(interior at row 1, col 2) so 3x3 taps become strided-view reads. A
second silu(x) copy shifted one column (interior at col 3) keeps every
depthwise tap 4-byte aligned for the DVE 2x bf16 mode.
"""
import sys

sys.path.insert(0, "/opt/trn_rl_repo")

import numpy as np
import ml_dtypes
from contextlib import ExitStack

import concourse.bass as bass
import concourse.mybir as mybir
from concourse import bacc, tile

F32 = mybir.dt.float32
BF16 = mybir.dt.bfloat16
AF = mybir.ActivationFunctionType
ALU = mybir.AluOpType

H = W = 64
HW = H * W  # 4096
PR = 66  # padded rows
PC = 68  # padded cols
PN = PR * PC  # 4488
NCHUNK = 8
CH = 512  # chunk size (positions)
NCI = 4  # 512-ch input tiles
NCO = 4  # 512-ch output tiles
NIMG = 2  # images per core
EPS = 1e-5

_cache = {}


def _pad_view(t, row0, col0, nrows=8):
    """[128, nrows x 64] view of padded tile starting (row0, col0)."""
    return bass.AP(
        tensor=t.tensor,
        offset=t.offset + row0 * PC + col0,
        ap=[t.ap[0], [PC, nrows], [1, W]],
    )


def _interior(t, col0):
    """[128, 64, 64] interior view (rows 1..64, cols col0..col0+63)."""
    return bass.AP(
        tensor=t.tensor,
        offset=t.offset + 1 * PC + col0,
        ap=[t.ap[0], [PC, H], [1, W]],
    )


def build():
    nc = bacc.Bacc()
    P = 128

    x_d = nc.declare_dram_parameter("x", [NIMG, NCI, P, HW], BF16, isOutput=False)
    pw_d = nc.declare_dram_parameter("pw", [P, NCI, NCO, P], BF16, isOutput=False)
    inner_d = nc.declare_dram_parameter("inner", [P, NCI, P], BF16, isOutput=False)
    outw_d = nc.declare_dram_parameter("outw", [P, NCO, P], BF16, isOutput=False)
    poly_d = nc.declare_dram_parameter("poly", [P, 9, 3, P], BF16, isOutput=False)
    dwt_d = nc.declare_dram_parameter("dwt", [P, NCI, 9], F32, isOutput=False)
    y0_d = nc.declare_dram_parameter("y0", [P, HW], BF16, isOutput=False)
    gnb_d = nc.declare_dram_parameter("gnb", [P, NCO, 2], F32, isOutput=False)
    bts_d = nc.declare_dram_parameter("bts", [P, 3], F32, isOutput=False)
    out_d = nc.declare_dram_parameter("out", [NIMG, NCO, P, HW], F32, isOutput=True)

    with ExitStack() as ctx:
        tc = ctx.enter_context(tile.TileContext(nc))
        wpool = ctx.enter_context(tc.tile_pool(name="w", bufs=1))
        ppool = ctx.enter_context(tc.tile_pool(name="pad", bufs=1))
        big = ctx.enter_context(tc.tile_pool(name="big", bufs=4))
        small = ctx.enter_context(tc.tile_pool(name="small", bufs=1))
        bbp = ctx.enter_context(tc.tile_pool(name="bb", bufs=2))
        yp = ctx.enter_context(tc.tile_pool(name="y", bufs=2))
        stp = ctx.enter_context(tc.tile_pool(name="st", bufs=3))
        psi = ctx.enter_context(tc.tile_pool(name="psi", bufs=2, space="PSUM"))
        psp = ctx.enter_context(tc.tile_pool(name="psp", bufs=2, space="PSUM"))
        psb = ctx.enter_context(tc.tile_pool(name="psb", bufs=4, space="PSUM"))

        # ---- weights / constants (loaded once) ----
        pw_w = wpool.tile([P, NCI, NCO, P], BF16)
        inner_w = wpool.tile([P, NCI, P], BF16)
        outw_w = wpool.tile([P, NCO, P], BF16)
        poly_w = wpool.tile([P, 9, 3, P], BF16)
        dwt = wpool.tile([P, NCI, 9], F32)
        y0 = wpool.tile([P, HW], BF16)
        gnb = wpool.tile([P, NCO, 2], F32)
        bts = wpool.tile([P, 3], F32)
        nc.sync.dma_start(out=pw_w, in_=pw_d[:, :, :, :])
        nc.sync.dma_start(out=inner_w, in_=inner_d[:, :, :])
        nc.sync.dma_start(out=outw_w, in_=outw_d[:, :, :])
        nc.sync.dma_start(out=poly_w, in_=poly_d[:, :, :, :])
        nc.sync.dma_start(out=dwt, in_=dwt_d[:, :, :])
        nc.sync.dma_start(out=y0, in_=y0_d[:, :])
        nc.sync.dma_start(out=gnb, in_=gnb_d[:, :, :])
        nc.sync.dma_start(out=bts, in_=bts_d[:, :])

        # ---- persistent padded buffers (borders zeroed once) ----
        sp = [ppool.tile([P, PN], BF16, name=f"sp{t}") for t in range(NCI)]
        spo = [ppool.tile([P, PN], BF16, name=f"spo{t}") for t in range(NCI)]
        gb = [ppool.tile([P, PN], BF16, name=f"gb{j}") for j in range(3)]
        for t in sp + spo + gb:
            nc.vector.memset(t, 0.0)

        xi = small.tile([P, HW], BF16, name="xi")
        xisq = small.tile([P, HW], BF16, name="xisq")
        stats = small.tile([P, NCO, NCHUNK, 6], F32, name="stats")
        mvs = small.tile([P, NCO, 2], F32, name="mvs")
        scb = small.tile([P, NCO, 3], F32, name="scb")  # scale, negscale, bias
        sqv = small.tile([P, NCO, 1], F32, name="sqv")

        for s in range(NIMG):
            # ---- load x ----
            xt = [big.tile([P, HW], BF16, tag="big", name=f"xt{s}_{t}") for t in range(NCI)]
            for t in range(NCI):
                nc.sync.dma_start(out=xt[t], in_=x_d[s, t, :, :])

            # ---- inner conv (512->128) + tanh -> xi ----
            for n in range(NCHUNK):
                ps = psi.tile([P, CH], F32, tag="psi", name=f"psi{s}_{n}")
                for t in range(NCI):
                    nc.tensor.matmul(
                        ps,
                        inner_w[:, t, :],
                        xt[t][:, n * CH : (n + 1) * CH],
                        start=(t == 0),
                        stop=(t == NCI - 1),
                    )
                nc.scalar.activation(xi[:, n * CH : (n + 1) * CH], ps, AF.Tanh)

            # ---- silu(x) into padded sp (col 2) and spo (col 3) ----
            for t in range(NCI):
                xv = xt[t].rearrange("p (a b) -> p a b", a=H)
                nc.scalar.activation(_interior(sp[t], 2), xv, AF.Silu)
                nc.scalar.activation(_interior(spo[t], 3), xv, AF.Silu)

            # ---- gram basis -> gb (padded, col 2) ----
            xiv = xi.rearrange("p (a b) -> p a b", a=H)
            nc.scalar.activation(_interior(gb[0], 2), xiv, AF.Silu)
            nc.vector.tensor_mul(xisq, xi, xi)
            nc.scalar.activation(
                _interior(gb[1], 2),
                xisq.rearrange("p (a b) -> p a b", a=H),
                AF.Silu,
                bias=bts[:, 0:1],
            )
            # xisq <- (xisq - bt2 - bt3) * xi  (= p3), in place
            nc.vector.scalar_tensor_tensor(
                xisq, xisq, bts[:, 1:2], xi, ALU.add, ALU.mult
            )
            nc.scalar.activation(
                _interior(gb[2], 2),
                xisq.rearrange("p (a b) -> p a b", a=H),
                AF.Silu,
            )

            # ---- accumulator tiles (reuse x slots) ----
            acc = [big.tile([P, HW], BF16, tag="big", name=f"acc{s}_{t}") for t in range(NCO)]

            # ---- main chunk loop ----
            for n in range(NCHUNK):
                h0 = n * 8
                # depthwise conv -> bb (bf16 chunks)
                bb = [bbp.tile([P, CH], BF16, tag=f"bb{t}", name=f"bb{s}_{n}_{t}") for t in range(NCI)]
                for t in range(NCI):
                    bv = bb[t].rearrange("p (a b) -> p a b", a=8)
                    first = True
                    for ky in range(3):
                        for kx in range(3):
                            tap = ky * 3 + kx
                            if kx == 1:
                                src = _pad_view(sp[t], h0 + ky, 2)
                            else:
                                src = _pad_view(spo[t], h0 + ky, kx + 2)
                            w_ap = dwt[:, t, tap : tap + 1]
                            if first:
                                nc.vector.tensor_scalar_mul(bv, src, w_ap)
                                first = False
                            else:
                                nc.vector.scalar_tensor_tensor(
                                    bv, src, w_ap, bv, ALU.mult, ALU.add
                                )

                # pointwise conv accumulation into basis banks
                pb = [psb.tile([P, CH], F32, tag="psb", name=f"pb{s}_{n}_{t}") for t in range(NCO)]
                for co in range(NCO):
                    for ci in range(NCI):
                        nc.tensor.matmul(
                            pb[co], pw_w[:, ci, co, :], bb[ci], start=(ci == 0), stop=False
                        )

                # poly 3x3 conv (gram tiles 1..3; tile 0 folded into y0)
                pp = psp.tile([P, CH], F32, tag="psp", name=f"psp{s}_{n}")
                idx = 0
                for ky in range(3):
                    for kx in range(3):
                        tap = ky * 3 + kx
                        for j in range(3):
                            nc.tensor.matmul(
                                pp,
                                poly_w[:, tap, j, :],
                                _pad_view(gb[j], h0 + ky, kx + 1),
                                start=(idx == 0),
                                stop=(idx == 26),
                            )
                            idx += 1

                # y = poly psum + y0 (const-tile contribution)
                y_sb = yp.tile([P, CH], BF16, tag="y", name=f"y{s}_{n}")
                nc.vector.tensor_add(y_sb, pp, y0[:, n * CH : (n + 1) * CH])

                # out conv accumulates onto basis banks -> y + basis
                for co in range(NCO):
                    nc.tensor.matmul(
                        pb[co], outw_w[:, co, :], y_sb, start=False, stop=True
                    )

                # drain: acc (bf16) + per-chunk stats from fp32 psum
                for co in range(NCO):
                    nc.scalar.activation(
                        acc[co][:, n * CH : (n + 1) * CH], pb[co], AF.Copy
                    )
                    nc.vector.bn_stats(out=stats[:, co, n, :], in_=pb[co])

            # ---- instance norm params ----
            for co in range(NCO):
                nc.vector.bn_aggr(out=mvs[:, co, :], in_=stats[:, co, :, :])
            for co in range(NCO):
                nc.scalar.activation(sqv[:, co, :], mvs[:, co, 1:2], AF.Sqrt, bias=bts[:, 2:3])
            for co in range(NCO):
                nc.vector.reciprocal(sqv[:, co, :], sqv[:, co, :])
                nc.vector.tensor_mul(scb[:, co, 0:1], sqv[:, co, :], gnb[:, co, 0:1])
                nc.vector.tensor_scalar_mul(scb[:, co, 1:2], scb[:, co, 0:1], -1.0)
                nc.vector.scalar_tensor_tensor(
                    scb[:, co, 2:3],
                    mvs[:, co, 0:1],
                    scb[:, co, 1:2],
                    gnb[:, co, 1:2],
                    ALU.mult,
                    ALU.add,
                )

            # ---- final silu(norm) -> DRAM ----
            for co in range(NCO):
                for n in range(NCHUNK):
                    st = stp.tile([P, CH], F32, tag="st", name=f"st{s}_{co}_{n}")
                    nc.scalar.activation(
                        st,
                        acc[co][:, n * CH : (n + 1) * CH],
                        AF.Silu,
                        scale=scb[:, co, 0:1],
                        bias=scb[:, co, 2:3],
                    )
                    nc.sync.dma_start(
                        out=out_d[s, co, :, n * CH : (n + 1) * CH], in_=st
                    )

    nc.compile()
    return nc


def _prep_weights(dw_w, pw_w, inner_w, out_w, gamma, beta_p, poly_w, beta_w):
    bf = ml_dtypes.bfloat16
    P = 128
    pw = pw_w[0, :, :, 0, 0]  # [O=512, I=512]
    # [P_k, NCI, NCO, P_m]: pw_l[k, ci, co, m] = pw[co*128+m, ci*128+k]
    pw_l = np.ascontiguousarray(
        pw.reshape(NCO, P, NCI, P).transpose(3, 2, 0, 1)
    )
    inner = inner_w[0, :, :, 0, 0]  # [128, 512]
    # [P_k, NCI, P_m]
    inner_l = np.ascontiguousarray(inner.reshape(P, NCI, P).transpose(2, 1, 0))
    outw = out_w[0, :, :, 0, 0]  # [512, 128]
    # [P_k, NCO, P_m]
    outw_l = np.ascontiguousarray(outw.reshape(NCO, P, P).transpose(2, 0, 1))
    poly = poly_w[0]  # [128, 512, 3, 3]
    # [P_k, 9, 3, P_m]: poly_l[k, ky*3+kx, j, m] = poly[m, (j+1)*128+k, ky, kx]
    poly_l = np.ascontiguousarray(
        poly.reshape(P, NCI, P, 3, 3)[:, 1:, :, :, :]
        .transpose(2, 3, 4, 1, 0)
        .reshape(P, 9, 3, P)
    )
    # [P, NCI, 9]
    dwt = np.ascontiguousarray(
        dw_w[0, :, 0, :, :].reshape(NCI, P, 9).transpose(1, 0, 2)
    )

    # y0: poly-conv contribution of the constant gram tile silu(1)*ones
    s1 = 1.0 / (1.0 + np.exp(-1.0))
    A = poly[:, 0:P, :, :].sum(axis=1)  # [128, 3, 3]
    ones_pad = np.zeros((H + 2, W + 2), np.float32)
    ones_pad[1:-1, 1:-1] = 1.0
    y0 = np.zeros((P, H, W), np.float32)
    for ky in range(3):
        for kx in range(3):
            y0 += A[:, ky, kx][:, None, None] * ones_pad[ky : ky + H, kx : kx + W]
    y0 *= s1

    bt2 = 2.25 * float(beta_w[1])
    bt3 = (100.0 / 3.0) * float(beta_w[2])
    bts = np.tile(np.array([[-bt2, -(bt2 + bt3), EPS]], np.float32), (P, 1))

    # [P, NCO, 2]
    gnb = np.stack(
        [gamma[0].reshape(NCO, P).T, beta_p[0].reshape(NCO, P).T], axis=2
    ).astype(np.float32)

    return {
        "pw": pw_l.astype(bf),
        "inner": inner_l.astype(bf),
        "outw": outw_l.astype(bf),
        "poly": poly_l.astype(bf),
        "dwt": np.ascontiguousarray(dwt, np.float32),
        "y0": y0.reshape(P, HW).astype(bf),
        "gnb": np.ascontiguousarray(gnb),
        "bts": bts,
    }


def _run(inputs, trace=False):
    from concourse.bass_utils import run_bass_kernel_spmd

    if "nc" not in _cache:
        _cache["nc"] = build()
    nc = _cache["nc"]

    rgb, ir = inputs["rgb"], inputs["ir"]
    bs = rgb.shape[0]
    wmap = _prep_weights(
        inputs["dw_w"],
        inputs["pw_w"],
        inputs["inner_w"],
        inputs["out_w"],
        inputs["gamma"],
        inputs["beta_p"],
        inputs["poly_w"],
        inputs["beta_w"],
    )
    bf = ml_dtypes.bfloat16
    in_maps = []
    for i in range(bs):
        x = np.empty((NIMG, NCI, 128, HW), bf)
        for s, feat in enumerate((rgb, ir)):
            xt = np.asarray(feat[i], np.float32).T  # [512, 4096]
            x[s] = xt.reshape(NCI, 128, HW).astype(bf)
        in_maps.append({"x": x, **wmap})

    res = run_bass_kernel_spmd(nc, in_maps, list(range(8)), trace=trace)

    rgb_out = np.empty((bs, HW, 512), np.float32)
    ir_out = np.empty((bs, HW, 512), np.float32)
    for i in range(bs):
        o = res.results[i]["out"]  # [2, 4, 128, 4096] f32
        rgb_out[i] = o[0].reshape(512, HW).T
        ir_out[i] = o[1].reshape(512, HW).T
    return (rgb_out, ir_out), res


def kernel(**inputs):
    out, _ = _run(inputs, trace=False)
    return out
